# revision 30
# baseline (speedup 1.0000x reference)
"""PIoU (pixel-wise IoU) pairwise matrix kernel for Trainium2, 8 NeuronCores.

Math: for each pair (predicted box n, target box m) the reference samples a
16x16 grid of the joint AABB and evaluates a soft membership
F = sigmoid(k(w/2-|A|)) * sigmoid(k(h/2-|B|)) per box, where (A, B) are the
pixel offsets rotated into the box frame.  Both A and B are *affine* in the
grid coordinates (ug, uh), so the sigmoid arguments (s/2 -+ A) for all
256 pixels x 4 fields x {P,Q} come from ONE K=32 matmul per (n, 128-m) tile
against a constant basis.  sigmoid(k*min(P,Q)) == the reference membership
factor, with the slope k folded into the activation's free scale field.

Since P + Q = s >= 8 and k = 10, the larger of sigmoid(kP), sigmoid(kQ) is
>= sigmoid(40) = 1 - 4e-18, so sigmoid(k*min(P,Q)) == sigmoid(kP)*sigmoid(kQ)
to machine precision -- the membership factor is a plain product, no min
needed (a VE min would need two PSUM reads, which the HW forbids).

Per (n, m-chunk) pipeline:
  PE   : 4 fp16 matmuls  [32,128]x[32,512] -> PSUM [128, 2048] (P|Q blocks)
  ACT  : sig  = Sigmoid(K * PQ)            [128, 2048] bf16 (one instruction)
  DVE  : G    = sigP * sigQ                [128, 1024] (field memberships)
  DVE  : TTR  Fp  = gA*gB, accum -> S      [128, 512]
  DVE  : TTR  F12 = F1*F2, accum -> I      [128, 256]
Coefficient slabs are built on the (otherwise idle) GPSIMD engine; lhsT
tiles come from batched PE transposes (4 n per transpose, 32-row bands via
matmul tile_position).

Sharding: N (predicted) axis split 8 ways; each core computes a [512m, 64n]
slab (output transposed on host).  Dispatch uses a persistent jitted
shard_map callable so steady-state calls skip jax re-trace/re-lowering.
"""

import numpy as np

N = 512
M = 512
G = 16
NPIX = G * G
K_SLOPE = np.float32(10.0)
EPS = np.float32(1e-6)
NC = 8
NLOC = N // NC  # 64 predicted boxes per core
NCHUNK = 4  # m-chunks of 128
KR = 32  # coefficient rows (24 used + 8 zero pad, for 32-aligned PE row bands)

_cache = {}

# build-time feature toggles (for hardware bisection)
USE_BANDS = True       # 32-row PE bands via matmul tile_position
USE_WARMUP = True      # PE warmup matmul on the BASIS DMA semaphore
SPLIT_ACT = False      # two 1024-col ACTIVATEs instead of one 2048-col
USE_TTR = False        # fused tensor_tensor_reduce -- HANGS TRN2 HW, keep off
USE_FP16 = True        # fp16 matmul operands (else fp32)
ACT_COPY = True        # stash copy on scalar engine (else vector)
GP_REDUCE = False      # GPSIMD tensor_reduce can't do free-axis sums; keep off
MOVING_BF16 = True     # basis (moving operand) in bf16 -- exact values
STASH_BF16 = False     # lhsT stays fp16: PE rate is pstate-bound, not dtype
GP_MULS = True         # Fp/F12 on GPSIMD + 1-n-delayed VE reduces

_QORDER = ("x0", "x1", "y0", "y1", "cx", "cy", "ct", "st", "shw", "shh")


def _derived(b):
    # b: [K,5] float32 -> per-box derived quantities (all float32)
    cx, cy, w, h, t = (b[:, i].astype(np.float32) for i in range(5))
    c, s = np.cos(t).astype(np.float32), np.sin(t).astype(np.float32)
    hw = np.float32(0.5) * (w * np.abs(c) + h * np.abs(s))
    hh = np.float32(0.5) * (w * np.abs(s) + h * np.abs(c))
    return dict(
        cx=cx, cy=cy, ct=c, st=s,
        shw=np.float32(0.5) * w, shh=np.float32(0.5) * h,
        x0=cx - hw, x1=cx + hw, y0=cy - hh, y1=cy + hh,
    )


def _basis():
    # [128, 2048] fp16: four replicated 32-row blocks (one per PE row band).
    # Within a block: P cols 0..1023, Q cols 1024..2047; field f at
    # cols f*256..(f+1)*256 uses rows 3f..3f+2 (P) / 12+3f.. (Q) = (1, Ug, Uh).
    u = (np.arange(G, dtype=np.float32) + np.float32(0.5)) / np.float32(G)
    Ug = np.tile(u, G)      # pixel p = h*G+g -> u[g]
    Uh = np.repeat(u, G)    # -> u[h]
    bas = np.zeros((KR, 8 * NPIX), dtype=np.float16)
    for f in range(4):
        for blk, r0 in ((0, 0), (1, 12)):
            c0 = blk * 4 * NPIX + f * NPIX
            bas[r0 + 3 * f + 0, c0:c0 + NPIX] = 1.0
            bas[r0 + 3 * f + 1, c0:c0 + NPIX] = Ug
            bas[r0 + 3 * f + 2, c0:c0 + NPIX] = Uh
    return np.vstack([bas] * 4)  # [128, 2048]


def _host_constants(loc_p, loc_t):
    """Build per-core input arrays (all O(N+M) host work)."""
    T = _derived(loc_t)
    # TQ [128, 4 chunks, 10]: per-target quantities, m = j*128 + partition
    TQ = np.empty((128, NCHUNK, len(_QORDER)), dtype=np.float32)
    for qi, q in enumerate(_QORDER):
        TQ[:, :, qi] = T[q].reshape(NCHUNK, 128).T

    P = _derived(loc_p)
    PBs = []
    for c in range(NC):
        sl = slice(c * NLOC, (c + 1) * NLOC)
        pb = np.stack([P[q][sl] for q in _QORDER], axis=0)  # [10, 64]
        PBs.append(np.broadcast_to(pb.reshape(1, 10 * NLOC), (128, 10 * NLOC)).copy())
    return _basis(), TQ.reshape(128, NCHUNK * len(_QORDER)), PBs


def _build_nc():
    from contextlib import ExitStack

    import concourse.bacc as bacc
    import concourse.tile as tile
    from concourse import mybir
    from concourse.masks import make_identity

    dt = mybir.dt
    op = mybir.AluOpType
    AF = mybir.ActivationFunctionType
    K = float(K_SLOPE)

    # Bacc (not raw Bass): its finalize() runs generate_event_semaphores,
    # which legalizes Tile's multi-wait sync_info down to <=1 wait per
    # hardware instruction.
    nc = bacc.Bacc(None, target_bir_lowering=False)
    PB_d = nc.declare_dram_parameter("PB", [128, 10 * NLOC], dt.float32, isOutput=False)
    TQ_d = nc.declare_dram_parameter("TQ", [128, NCHUNK * 10], dt.float32, isOutput=False)
    mmdt = (dt.bfloat16 if STASH_BF16 else dt.float16) if USE_FP16 else dt.float32
    basdt = dt.bfloat16 if (USE_FP16 and MOVING_BF16) else mmdt
    BAS_d = nc.declare_dram_parameter("BASIS", [128, 8 * NPIX], basdt, isOutput=False)
    OUT_d = nc.declare_dram_parameter("OUT", [M, NLOC], dt.float32, isOutput=True)

    with tile.TileContext(nc) as tc, ExitStack() as ctx:
        consts = ctx.enter_context(tc.tile_pool(name="consts", bufs=1))
        stashp = ctx.enter_context(tc.tile_pool(name="stashp", bufs=2))
        vminp = ctx.enter_context(tc.tile_pool(name="vminp", bufs=3))
        sigp = ctx.enter_context(tc.tile_pool(name="sigp", bufs=3))
        fpp = ctx.enter_context(tc.tile_pool(name="fpp", bufs=3))
        accp = ctx.enter_context(tc.tile_pool(name="accp", bufs=2))
        psum = ctx.enter_context(tc.tile_pool(name="psum", bufs=2, space="PSUM"))

        ident = consts.tile([128, 128], dt.float32)
        make_identity(nc, ident[:])
        PB = consts.tile([128, 10, NLOC], dt.float32)
        nc.sync.dma_start(out=PB[:].rearrange("p a b -> p (a b)"), in_=PB_d[:])
        TQ = consts.tile([128, NCHUNK, 10], dt.float32)
        nc.sync.dma_start(out=TQ[:].rearrange("p a b -> p (a b)"), in_=TQ_d[:])
        BAS = consts.tile([128, 8 * NPIX], basdt)
        nc.sync.dma_start(out=BAS[:], in_=BAS_d[:])

        # Coefficient slab, n-major so 4 consecutive n flatten to one
        # [128, 128] transpose input.  Rows 24..31 stay zero (PE K pad).
        C = consts.tile([128, NLOC, KR], dt.float32)
        S = consts.tile([128, 16, NLOC], dt.float32)

        def pb(q):
            return PB[:, _QORDER.index(q), :]

        def tq(j, q):
            i = _QORDER.index(q)
            return TQ[:, j, i:i + 1]

        # GPSIMD can't run TensorScalarPtr (per-partition AP scalar), which
        # the t-box ops need, so the coefficient phase runs on DVE.
        g = nc.vector
        nc.gpsimd.memset(C[:, :, 24:KR], 0.0)

        def s(i):
            return S[:, i, :]

        for j in range(NCHUNK):
            # ---- coefficient slab C [128 m, 64 n, 32 rows] on GPSIMD ----
            def c(r):
                return C[:, :, r]

            if j == 0:
                # Engine instructions carry a single HW sync-wait slot, so
                # the first op after the two input DMAs may not wait on both
                # DMA sems at once.  Chain two single-wait ops; the WAW
                # overlap with s(0) orders the real first op after them.
                g.tensor_copy(s(0)[:, 1:2], PB[:, 0, 0:1])
                g.tensor_copy(s(0)[:, 0:1], TQ[:, 0, 0:1])

            g.tensor_scalar(s(0), pb("x0"), tq(j, "x0"), None, op.min)   # xmin
            g.tensor_scalar(s(1), pb("x1"), tq(j, "x1"), None, op.max)   # xmax
            g.tensor_scalar(s(2), pb("y0"), tq(j, "y0"), None, op.min)   # ymin
            g.tensor_scalar(s(3), pb("y1"), tq(j, "y1"), None, op.max)   # ymax
            g.tensor_tensor(s(4), s(1), s(0), op.subtract)               # sx
            g.tensor_tensor(s(5), s(3), s(2), op.subtract)               # sy
            g.tensor_tensor(s(6), s(0), pb("cx"), op.subtract)           # dxp
            g.tensor_tensor(s(7), s(2), pb("cy"), op.subtract)           # dyp
            # a0p = dxp*ctp + dyp*stp ; b0p = dyp*ctp - dxp*stp
            g.tensor_tensor(s(8), s(6), pb("ct"), op.mult)
            g.tensor_tensor(s(9), s(7), pb("st"), op.mult)
            g.tensor_tensor(s(9), s(8), s(9), op.add)                    # a0p
            g.tensor_tensor(s(8), s(7), pb("ct"), op.mult)
            g.tensor_tensor(s(10), s(6), pb("st"), op.mult)
            g.tensor_tensor(s(10), s(8), s(10), op.subtract)             # b0p

            # field A1: P = shw_p - a0p (const row), Q = shw_p + a0p
            g.scalar_tensor_tensor(c(0), s(9), -1.0, pb("shw"), op.mult, op.add)
            g.scalar_tensor_tensor(c(12), s(9), 1.0, pb("shw"), op.mult, op.add)
            # a1p = sx*ctp -> rows 1/13 ; a2p = sy*stp -> rows 2/14
            g.tensor_tensor(s(8), s(4), pb("ct"), op.mult)
            g.tensor_scalar(c(1), s(8), -1.0, None, op.mult)
            g.tensor_copy(c(13), s(8))
            g.tensor_tensor(s(8), s(5), pb("st"), op.mult)
            g.tensor_scalar(c(2), s(8), -1.0, None, op.mult)
            g.tensor_copy(c(14), s(8))
            # field B1 (rows 6-8/18-20; field order is A1,A2,B1,B2)
            g.scalar_tensor_tensor(c(6), s(10), -1.0, pb("shh"), op.mult, op.add)
            g.scalar_tensor_tensor(c(18), s(10), 1.0, pb("shh"), op.mult, op.add)
            # b1p = -sx*stp: P row = +sx*stp, Q row = -sx*stp
            g.tensor_tensor(s(8), s(4), pb("st"), op.mult)
            g.tensor_copy(c(7), s(8))
            g.tensor_scalar(c(19), s(8), -1.0, None, op.mult)
            # b2p = sy*ctp
            g.tensor_tensor(s(8), s(5), pb("ct"), op.mult)
            g.tensor_scalar(c(8), s(8), -1.0, None, op.mult)
            g.tensor_copy(c(20), s(8))
            # target box: dxt/dyt
            g.tensor_scalar(s(12), s(0), tq(j, "cx"), None, op.subtract)
            g.tensor_scalar(s(13), s(2), tq(j, "cy"), None, op.subtract)
            # a0t = dxt*ctt + dyt*stt
            g.tensor_scalar(s(8), s(12), tq(j, "ct"), None, op.mult)
            g.tensor_scalar(s(14), s(13), tq(j, "st"), None, op.mult)
            g.tensor_tensor(s(14), s(8), s(14), op.add)
            # b0t = dyt*ctt - dxt*stt
            g.tensor_scalar(s(8), s(13), tq(j, "ct"), None, op.mult)
            g.tensor_scalar(s(15), s(12), tq(j, "st"), None, op.mult)
            g.tensor_tensor(s(15), s(8), s(15), op.subtract)
            # field A2 const rows (rows 3-5/15-17)
            g.tensor_scalar(c(3), s(14), -1.0, tq(j, "shw"), op.mult, op.add)
            g.tensor_scalar(c(15), s(14), 1.0, tq(j, "shw"), op.mult, op.add)
            # a1t = sx*ctt ; a2t = sy*stt
            g.tensor_scalar(s(8), s(4), tq(j, "ct"), None, op.mult)
            g.tensor_scalar(c(4), s(8), -1.0, None, op.mult)
            g.tensor_copy(c(16), s(8))
            g.tensor_scalar(s(8), s(5), tq(j, "st"), None, op.mult)
            g.tensor_scalar(c(5), s(8), -1.0, None, op.mult)
            g.tensor_copy(c(17), s(8))
            # field B2 const rows
            g.tensor_scalar(c(9), s(15), -1.0, tq(j, "shh"), op.mult, op.add)
            g.tensor_scalar(c(21), s(15), 1.0, tq(j, "shh"), op.mult, op.add)
            # b1t = -sx*stt ; b2t = sy*ctt
            g.tensor_scalar(s(8), s(4), tq(j, "st"), None, op.mult)
            g.tensor_copy(c(10), s(8))
            g.tensor_scalar(c(22), s(8), -1.0, None, op.mult)
            g.tensor_scalar(s(8), s(5), tq(j, "ct"), None, op.mult)
            g.tensor_scalar(c(11), s(8), -1.0, None, op.mult)
            g.tensor_copy(c(23), s(8))

            Ssum = accp.tile([128, NLOC], dt.float32, tag="Ssum")
            Isum = accp.tile([128, NLOC], dt.float32, tag="Isum")
            if USE_BANDS:
                stash = stashp.tile([128, 16, 128], mmdt, tag="stash")
            else:
                stash = stashp.tile([32, NLOC, 128], mmdt, tag="stash")

            # ---- main loop: 16 groups of 4 n ----
            # With GP_MULS, the products run on GPSIMD and the VE reduces for
            # iteration n are emitted during iteration n+1, so the VE never
            # stalls on the cross-engine Fp/F12 handoff.
            pending = None  # (n, Fp, F12) awaiting reduces

            def flush_pending():
                nonlocal pending
                if pending is None:
                    return
                pn, pFp, pF12 = pending
                nc.vector.tensor_reduce(
                    Ssum[:, pn:pn + 1], pFp[:], mybir.AxisListType.X, op.add)
                nc.vector.tensor_reduce(
                    Isum[:, pn:pn + 1], pF12[:], mybir.AxisListType.X, op.add)
                pending = None

            for grp in range(NLOC // 4):
                if USE_BANDS:
                    # lhsT for 4 n at once: C[:, 4g:4g+4, :] flattens to
                    # [128, 128]; transpose -> [128(4 bands x 32 rows), 128].
                    T = psum.tile([128, 8 * NPIX], dt.float32, tag="pq")
                    if USE_WARMUP and j == 0 and grp == 0:
                        # Warm the PE clock on the BAS DMA sem (single-wait
                        # LDW) before the first real transpose, which must
                        # wait on the C coefficients.  WAW into T orders it.
                        nc.tensor.matmul(
                            T[:, 0:512], BAS[0:32, 0:128], BAS[0:32, 0:512],
                            start=True, stop=True)
                    nc.tensor.transpose(
                        T[:, 0:128],
                        C[:, 4 * grp:4 * grp + 4, :].rearrange("p n r -> p (n r)"),
                        ident[:])
                    if ACT_COPY:
                        nc.scalar.copy(stash[:, grp, :], T[:, 0:128])
                    else:
                        nc.vector.tensor_copy(stash[:, grp, :], T[:, 0:128])
                else:
                    for k in range(4):
                        n = 4 * grp + k
                        T = psum.tile([128, 8 * NPIX], dt.float32, tag="pq")
                        if USE_WARMUP and j == 0 and n == 0:
                            nc.tensor.matmul(
                                T[:, 0:512], BAS[0:32, 0:128], BAS[0:32, 0:512],
                                start=True, stop=True)
                        nc.tensor.transpose(T[0:32, 0:128], C[:, n, :], ident[:])
                        if ACT_COPY:
                            nc.scalar.copy(stash[:, n, :], T[0:32, 0:128])
                        else:
                            nc.vector.tensor_copy(stash[:, n, :], T[0:32, 0:128])

                for k in range(4):
                    n = 4 * grp + k
                    PQ = psum.tile([128, 8 * NPIX], dt.float32, tag="pq")
                    for h in range(4):
                        if USE_BANDS:
                            nc.tensor.matmul(
                                PQ[:, h * 512:(h + 1) * 512],
                                stash[32 * k:32 * (k + 1), grp, :],
                                BAS[32 * k:32 * (k + 1), h * 512:(h + 1) * 512],
                                start=True, stop=True,
                                tile_position=(32 * k, 0))
                        else:
                            nc.tensor.matmul(
                                PQ[:, h * 512:(h + 1) * 512],
                                stash[:, n, :],
                                BAS[0:32, h * 512:(h + 1) * 512],
                                start=True, stop=True)
                    sig = sigp.tile([128, 8 * NPIX], dt.bfloat16, tag="sig")
                    if SPLIT_ACT:
                        nc.scalar.activation(
                            sig[:, 0:1024], PQ[:, 0:1024], AF.Sigmoid, 0.0, K)
                        nc.scalar.activation(
                            sig[:, 1024:2048], PQ[:, 1024:2048], AF.Sigmoid, 0.0, K)
                    else:
                        nc.scalar.activation(sig[:], PQ[:], AF.Sigmoid, 0.0, K)
                    Gm = vminp.tile([128, 4 * NPIX], dt.bfloat16, tag="Gm")
                    nc.vector.tensor_tensor(
                        Gm[:], sig[:, 0:1024], sig[:, 1024:2048], op.mult)
                    Fp = fpp.tile([128, 2 * NPIX], dt.bfloat16, tag="Fp")
                    F12 = fpp.tile([128, NPIX], dt.bfloat16, tag="F12")
                    if USE_TTR:
                        nc.vector.tensor_tensor_reduce(
                            Fp[:], Gm[:, 0:512], Gm[:, 512:1024],
                            1.0, 0.0, op.mult, op.add, Ssum[:, n:n + 1])
                        nc.vector.tensor_tensor_reduce(
                            F12[:], Fp[:, 0:NPIX], Fp[:, NPIX:2 * NPIX],
                            1.0, 0.0, op.mult, op.add, Isum[:, n:n + 1])
                    elif GP_MULS:
                        nc.gpsimd.tensor_mul(Fp[:], Gm[:, 0:512], Gm[:, 512:1024])
                        nc.gpsimd.tensor_mul(
                            F12[:], Fp[:, 0:NPIX], Fp[:, NPIX:2 * NPIX])
                        flush_pending()
                        pending = (n, Fp, F12)
                    else:
                        nc.vector.tensor_mul(Fp[:], Gm[:, 0:512], Gm[:, 512:1024])
                        nc.vector.tensor_reduce(
                            Ssum[:, n:n + 1], Fp[:], mybir.AxisListType.X, op.add)
                        nc.vector.tensor_mul(
                            F12[:], Fp[:, 0:NPIX], Fp[:, NPIX:2 * NPIX])
                        nc.vector.tensor_reduce(
                            Isum[:, n:n + 1], F12[:], mybir.AxisListType.X, op.add)

            # ---- epilogue: piou = inter / (stot - inter + eps) ----
            flush_pending()
            union = accp.tile([128, NLOC], dt.float32, tag="union")
            nc.vector.scalar_tensor_tensor(
                union[:], Isum[:], -1.0, Ssum[:], op.mult, op.add)
            nc.vector.tensor_scalar(union[:], union[:], float(EPS), None, op.add)
            rec = accp.tile([128, NLOC], dt.float32, tag="rec")
            nc.vector.reciprocal(rec[:], union[:])
            piou = accp.tile([128, NLOC], dt.float32, tag="piou")
            nc.vector.tensor_tensor(piou[:], Isum[:], rec[:], op.mult)
            nc.sync.dma_start(out=OUT_d[j * 128:(j + 1) * 128, :], in_=piou[:])

    nc.finalize()
    return nc


def _get_compiled():
    if "nc" not in _cache:
        _cache["nc"] = _build_nc()
    return _cache["nc"]


def _get_runner():
    """Persistent jitted shard_map callable (mirrors bass2jax.run_bass_via_pjrt
    but caches the traced/jitted function so steady-state calls skip jax
    re-trace + re-lowering, which dominate the per-call wall time)."""
    if "runner" in _cache:
        return _cache["runner"]

    import jax
    import numpy as _np
    from jax.experimental.shard_map import shard_map
    from jax.sharding import Mesh, PartitionSpec

    import concourse.bass2jax as b2j
    from concourse import mybir

    nc = _get_compiled()
    b2j.install_neuronx_cc_hook()
    partition_name = nc.partition_id_tensor.name if nc.partition_id_tensor else None

    in_names, out_names, out_avals, zero_shapes = [], [], [], []
    for alloc in nc.m.functions[0].allocations:
        if not isinstance(alloc, mybir.MemoryLocationSet):
            continue
        name = alloc.memorylocations[0].name
        if alloc.kind == "ExternalInput":
            if name != partition_name:
                in_names.append(name)
        elif alloc.kind == "ExternalOutput":
            out_names.append(name)
            shape = tuple(alloc.tensor_shape)
            dtype = mybir.dt.np(alloc.dtype)
            out_avals.append(jax.core.ShapedArray(shape, dtype))
            zero_shapes.append((shape, dtype))
    n_params = len(in_names)
    n_outs = len(out_avals)
    all_names = list(in_names) + list(out_names)
    if partition_name is not None:
        all_names.append(partition_name)
    donate = tuple(range(n_params, n_params + n_outs))

    def _body(*args):
        operands = list(args)
        if partition_name is not None:
            operands.append(b2j.partition_id_tensor())
        outs = b2j._bass_exec_p.bind(
            *operands,
            out_avals=tuple(out_avals),
            in_names=tuple(all_names),
            out_names=tuple(out_names),
            lowering_input_output_aliases=(),
            sim_require_finite=True,
            sim_require_nnan=True,
            nc=nc,
        )
        return tuple(outs)

    devices = jax.devices()[:NC]
    assert len(devices) >= NC, f"need {NC} devices, have {len(jax.devices())}"
    mesh = Mesh(_np.asarray(devices), ("core",))
    in_specs = (PartitionSpec("core"),) * (n_params + n_outs)
    out_specs = (PartitionSpec("core"),) * n_outs
    sharded = jax.jit(
        shard_map(_body, mesh=mesh, in_specs=in_specs, out_specs=out_specs,
                  check_rep=False),
        donate_argnums=donate,
        keep_unused=True,
    )

    def run(in_maps):
        concat_in = [
            np.concatenate([np.asarray(in_maps[c][nm]) for c in range(NC)], axis=0)
            for nm in in_names
        ]
        zeros = [np.zeros((NC * sh[0], *sh[1:]), dtp) for sh, dtp in zero_shapes]
        out_arrs = sharded(*concat_in, *zeros)
        return [
            {nm: np.asarray(out_arrs[i]).reshape(NC, *out_avals[i].shape)[c]
             for i, nm in enumerate(out_names)}
            for c in range(NC)
        ]

    _cache["runner"] = run
    return run


def kernel(loc_p, loc_t, grid):
    assert int(grid) == G
    loc_p = np.asarray(loc_p, dtype=np.float32)
    loc_t = np.asarray(loc_t, dtype=np.float32)
    basis, TQ, PBs = _host_constants(loc_p, loc_t)
    if not USE_FP16:
        basis = basis.astype(np.float32)
    elif MOVING_BF16:
        import ml_dtypes

        basis = basis.astype(ml_dtypes.bfloat16)
    in_maps = [{"PB": PBs[c], "TQ": TQ, "BASIS": basis} for c in range(NC)]

    try:
        res = _get_runner()(in_maps)
    except Exception:
        # Robust fallback: the stock (slower) dispatch path.
        from concourse.bass_utils import run_bass_kernel_spmd

        res = run_bass_kernel_spmd(
            _get_compiled(), in_maps, core_ids=list(range(NC))).results

    out = np.empty((N, M), dtype=np.float32)
    for c in range(NC):
        out[c * NLOC:(c + 1) * NLOC, :] = res[c]["OUT"].T
    return out
